# revision 1
# baseline (speedup 1.0000x reference)
"""CosineCrossAttention Trainium2 kernel (v3).

Math (per (b,t)):
    q = query @ Wq                      (N, D), heads head-major: d = h*48+dh
    k = kv @ Wk   (1, D);  v = kv @ Wv  (1, D)
    attn[n,h] = (q_h . k_h) / (|q_h||k_h|)
    out[n, dh*8+h] = attn[n,h] * v[h,dh];  out = out @ Wp + bp

Restructured:
    Wqk[d,t,h]  = (Wq @ (k_t masked per head))          attn_raw = query @ Wqk
    ss[n,h]     = sum_{d in head h} q[n,d]^2   (mask48 matmul on q^2)
    attn        = attn_raw * rsqrt(ss);  1/|k_h| folded into Wp_eff
    Wp_eff[h,:] = sum_d v_perm[d]*(d%8==h)*Wp[d,:]
    out         = attn @ Wp_eff + bp

Performance structure (HW-measured: matmul streams ~512 rows at ~2.4GHz per
instruction regardless of dtype; fp8 DoubleRow contracts 2 k-tiles per
instruction, i.e. its win is instruction count, not row rate; every matmul
pays its own LDWEIGHTS ~90-130ns; DMA issue costs ~640ns on an engine queue):
  - q feeds only the per-head norms, so the q-projection runs fp8e4m3:
    DR(c0,c1) + plain(c2) = 2 instructions per 128-col output chunk (6 vs 9
    bf16).  The ss mask-matmul is fp8 DR too (mask padded to 128 stationary
    cols: DR ldweights requires full-width (0,0) tiles).  attn_raw and the
    out-projection stay bf16 (fp8 there fails the 2e-2 error budget; fp8 on
    the norm path costs ~+7e-3).  The fp8 query copy ships from the host
    (gpsimd cast was 10.6us/half-t - 4x too slow).
  - Two-stage lagged tail: stage A (ss matmul + 1/x on DVE + sqrt on ACT ->
    att) lags the body by 1 group; stage B (out matmuls + evac + DMA) lags
    by 2, so the tensor queue never stalls on the scalar/vector tail chain.
  - Per group (512 cols): tensor 14 matmuls (6 qproj + 3 attn + 2 ss + 3
    out); scalar 3 squares (write fp8 directly) + sqrt (+ alternating osb
    evac); vector 1/ss + att-mult + 2-3 osb evacs.  PSUM: a 4-bank ring for
    the qproj chunks + ss, 2 banks for attn_raw, 2 for the out-projection.
  - Consts arrive as packed blobs (1 fp8 + 1 bf16 + bias DMA); qt loads are
    one DMA per half-t (3D AP), triple-buffered; output is written bf16
    (halves HBM writes; host casts back to f32).
  - Small preamble matmuls (k/v projections emitted directly in transposed
    (D,t) orientation - no transpose instructions - then wqk, wpe) run first
    and warm the PE p-state while the first q-projection's inputs stream in.

Sharding: data-parallel over B across the 8 cores (one batch element each).
HW exec ~150us/core vs 183.7us for the bf16 v1 baseline; rel err 1.3e-2
(gate 2e-2, fp64 reference).
"""

import sys

sys.path.insert(0, "/opt/trn_rl_repo")

from contextlib import ExitStack

import ml_dtypes
import numpy as np

import concourse.tile as tile
from concourse import bacc, mybir

F32 = mybir.dt.float32
BF16 = mybir.dt.bfloat16
FP8 = mybir.dt.float8e4
DR = mybir.MatmulPerfMode.DoubleRow

B, T, N, D, H, Dh = 8, 8, 2048, 384, 8, 48
P = 128
CH = D // P  # 3 chunks of the D dims
NG = 512  # n-group (one PSUM bank of f32)

FP8_QPROJ = True  # q-projection + ss in fp8 DoubleRow (False: all bf16)
USE_DIVIDE = True  # att = attn_raw / nrm on DVE (False: sqrt+rcp+mult)

# bf16 const blob offsets (in bf16 elements per partition)
_WQT = 0
_WK = _WQT + CH * D
_WVP = _WK + CH * D
_WP = _WVP + CH * D
_M48 = _WP + CH * D
_MV = _M48 + CH * H
_KVT = _MV + CH * H


def _cb16_total(t_dim):
    return _KVT + CH * t_dim


def build_nc(t_dim=T, n_dim=N):
    nc = bacc.Bacc("TRN2", target_bir_lowering=False, debug=False)

    nh = min(2 * NG, n_dim)  # columns per qt tile (half-t)
    gph = nh // NG  # groups per half
    nhalves = n_dim // nh
    groups = [
        (t, hf, gl) for t in range(t_dim) for hf in range(nhalves) for gl in range(gph)
    ]
    G = len(groups)

    qT = nc.dram_tensor("qT", [t_dim, D, n_dim], BF16, kind="ExternalInput").ap()
    qT8 = nc.dram_tensor("qT8", [t_dim, D, n_dim], FP8, kind="ExternalInput").ap()
    cb16_d = nc.dram_tensor("cb16", [P, _cb16_total(t_dim)], BF16, kind="ExternalInput").ap()
    c8_d = nc.dram_tensor("c8", [P, CH * D + CH * P], FP8, kind="ExternalInput").ap()
    bp_d = nc.dram_tensor("bp", [P, CH], F32, kind="ExternalInput").ap()
    outT = nc.dram_tensor("outT", [t_dim, D, n_dim], BF16, kind="ExternalOutput").ap()

    with tile.TileContext(nc) as tc, ExitStack() as ctx:
        consts = ctx.enter_context(tc.tile_pool(name="consts", bufs=1))
        qpool = ctx.enter_context(tc.tile_pool(name="qpool", bufs=3))
        q8pool = ctx.enter_context(tc.tile_pool(name="q8pool", bufs=3))
        work = ctx.enter_context(tc.tile_pool(name="work", bufs=2))
        qsqp = ctx.enter_context(tc.tile_pool(name="qsqp", bufs=4))
        nrmp = ctx.enter_context(tc.tile_pool(name="nrmp", bufs=2))
        attp = ctx.enter_context(tc.tile_pool(name="attp", bufs=4))
        osbp = ctx.enter_context(tc.tile_pool(name="osbp", bufs=3))
        pqp = ctx.enter_context(tc.tile_pool(name="pqp", bufs=4, space="PSUM"))
        parp = ctx.enter_context(tc.tile_pool(name="parp", bufs=2, space="PSUM"))
        pop = ctx.enter_context(tc.tile_pool(name="pop", bufs=2, space="PSUM"))
        dram = ctx.enter_context(tc.tile_pool(name="dram", bufs=1, space="DRAM"))

        # ---------- const DMAs (c8 + first qt first: q-proj starts early) ----
        cb16_t = consts.tile([P, _cb16_total(t_dim)], BF16, tag="cb16")
        nc.sync.dma_start(cb16_t, cb16_d)

        c8_t = consts.tile([P, CH * D + CH * P], FP8, tag="c8")
        nc.sync.dma_start(c8_t, c8_d)
        wq8v = c8_t[:, 0 : CH * D].rearrange("p (c d) -> p c d", c=CH)
        # m48 padded to 128 stationary columns (cols 8.. are zero): DoubleRow
        # ldweights is only valid for full-width (0,0) tiles.
        m48v = c8_t[:, CH * D :].rearrange("p (c m) -> p c m", c=CH)

        qt_tiles = {}

        def emit_qdma(h):
            t, hf = h
            sl = slice(hf * nh, (hf + 1) * nh)
            if FP8_QPROJ:
                qt8 = q8pool.tile([P, CH, nh], FP8, tag="qt8")
                nc.sync.dma_start(
                    qt8, qT8[t].rearrange("(c p) n -> p c n", p=P)[:, :, sl]
                )
            else:
                qt8 = None
            qt = qpool.tile([P, CH, nh], BF16, tag="qt")
            nc.sync.dma_start(qt, qT[t].rearrange("(c p) n -> p c n", p=P)[:, :, sl])
            qt_tiles[h] = (qt, qt8)

        emit_qdma((0, 0))

        def W(off, c, w, sub=None, subw=P):
            base = off + c * w
            if sub is None:
                return cb16_t[:, base : base + w]
            return cb16_t[:, base + sub * subw : base + sub * subw + subw]

        bp_sb = consts.tile([P, CH], F32, tag="bp")
        nc.sync.dma_start(bp_sb, bp_d)

        # ---------- per-group emission helpers ----------
        par_tiles = {}
        qsq_tiles = {}
        att_tiles = {}

        def emit_qproj(g):
            t, hf, gl = groups[g]
            qt, qt8 = qt_tiles[(t, hf)]
            qsl = slice(gl * NG, (gl + 1) * NG)
            par = parp.tile([40, NG], F32, tag="par")
            qsq = qsqp.tile([P, CH, NG], FP8 if FP8_QPROJ else BF16, tag="qsq")
            for co in range(CH):
                pqc = pqp.tile([P, NG], F32, tag="pq")
                if FP8_QPROJ:
                    nc.tensor.matmul(
                        pqc, wq8v[:, 0:2, co * P : (co + 1) * P], qt8[:, 0:2, qsl],
                        start=True, stop=False, perf_mode=DR, skip_group_check=True,
                    )
                    nc.tensor.matmul(
                        pqc, wq8v[:, 2, co * P : (co + 1) * P], qt8[:, 2, qsl],
                        start=False, stop=True, skip_group_check=True,
                    )
                nc.scalar.square(qsq[:, co, :], pqc)
            par_tiles[g] = par
            qsq_tiles[g] = qsq

        def emit_attn(g):
            t, hf, gl = groups[g]
            qt, _ = qt_tiles[(t, hf)]
            qsl = slice(gl * NG, (gl + 1) * NG)
            par = par_tiles[g]
            for c in range(CH):
                nc.tensor.matmul(
                    par[0:H, :], wqk[:, c, t, :], qt[:, c, qsl],
                    start=(c == 0), stop=(c == CH - 1),
                    tile_position=(0, 0),
                )

        def emit_stageA(g):
            par = par_tiles[g]
            qsq = qsq_tiles.pop(g)
            if FP8_QPROJ:
                sspq = pqp.tile([P, NG], F32, tag="pq")
                nc.tensor.matmul(
                    sspq, m48v[:, 0:2, :], qsq[:, 0:2, :],
                    start=True, stop=False, perf_mode=DR, skip_group_check=True,
                )
                nc.tensor.matmul(
                    sspq, m48v[:, 2, :], qsq[:, 2, :],
                    start=False, stop=True, skip_group_check=True,
                )
                ss = sspq[0:H, :]
            else:
                for c in range(CH):
                    nc.tensor.matmul(
                        par[32 : 32 + H, :], _m48_sb(c), qsq[:, c, :],
                        start=(c == 0), stop=(c == CH - 1),
                        tile_position=(0, 32),
                    )
                ss = par[32 : 32 + H, :]
            # att = attn_raw * sqrt(1/ss); 1/|k_h| is folded into Wp_eff
            rss = nrmp.tile([H, NG], F32, tag="rss")
            nc.vector.reciprocal_approx_fast(rss, ss)
            rs = nrmp.tile([H, NG], F32, tag="rs")
            nc.scalar.sqrt(rs, rss)
            att = attp.tile([H, NG], BF16, tag="att")
            nc.vector.tensor_tensor(att, par[0:H, :], rs, op=mybir.AluOpType.mult)
            par_tiles.pop(g)
            att_tiles[g] = att

        def emit_stageB(g):
            t, hf, gl = groups[g]
            sl = slice(hf * nh + gl * NG, hf * nh + (gl + 1) * NG)
            att = att_tiles.pop(g)
            osb = osbp.tile([P, CH, NG], BF16, tag="osb")
            for co in range(CH):
                po = pop.tile([P, NG], F32, tag="po")
                nc.tensor.matmul(
                    po, wpe_s[:, t, co * P : (co + 1) * P], att,
                    start=True, stop=True,
                )
                if co == 0 and g % 2 == 0:
                    # alternate the first evac between engines to balance load
                    nc.scalar.activation(
                        osb[:, co, :], po,
                        mybir.ActivationFunctionType.Identity,
                        bias=bp_sb[:, co : co + 1], scale=1.0,
                    )
                else:
                    nc.vector.tensor_tensor(
                        osb[:, co, :], po,
                        bp_sb[:, co : co + 1].to_broadcast((P, NG)),
                        op=mybir.AluOpType.add,
                    )
            dst = outT[t].rearrange("(c p) n -> p c n", p=P)[:, :, sl]
            if g == G - 1:
                for co in range(CH):
                    nc.sync.dma_start(dst[:, co, :], osb[:, co, :])
            else:
                nc.sync.dma_start(dst, osb)

        def _m48_sb(c):
            return W(_M48, c, H)

        # ---------- preamble: k/v projections, wqk, wpe ----------
        PRE = min(2, G)
        # k/v projections computed directly in transposed (D-part, t) form:
        # stationary = weight chunk, moving = kvT (8 rows) - no transposes.
        kT = consts.tile([P, CH, t_dim], BF16, tag="kT")
        vT = consts.tile([P, CH, t_dim], BF16, tag="vT")
        for co in range(CH):
            pk = pqp.tile([P, t_dim], F32, tag="pq")
            for c in range(CH):
                nc.tensor.matmul(
                    pk, W(_WK, c, D)[:, co * P : (co + 1) * P], W(_KVT, c, t_dim),
                    start=(c == 0), stop=(c == CH - 1),
                )
            nc.vector.tensor_copy(kT[:, co, :], pk)
            pv = pop.tile([P, t_dim], F32, tag="po")
            for c in range(CH):
                nc.tensor.matmul(
                    pv, W(_WVP, c, D)[:, co * P : (co + 1) * P], W(_KVT, c, t_dim),
                    start=(c == 0), stop=(c == CH - 1),
                )
            nc.vector.tensor_copy(vT[:, co, :], pv)

        # per-head k norms: rnkT[h, t] = 1/|k_h|(t)
        emit_qproj(0)
        if PRE > 1:
            emit_qproj(1)

        ksqT = work.tile([P, CH, t_dim], BF16, tag="ksqT")
        nc.scalar.square(ksqT, kT)
        psk2 = parp.tile([H, t_dim], F32, tag="par")
        for c in range(CH):
            nc.tensor.matmul(
                psk2, W(_M48, c, H), ksqT[:, c, :],
                start=(c == 0), stop=(c == CH - 1),
            )
        rnkT = consts.tile([H, t_dim], F32, tag="rnkT")
        nc.scalar.sqrt(rnkT, psk2)
        nc.vector.reciprocal(rnkT, rnkT)

        # Kmat[d, t, h] = kT[d, t] * m48[d, h];  Vsel[d, t, h] = vT[d, t] * mv[d, h]
        m48b = cb16_t[:, _M48:_MV].rearrange("p (c h) -> p c h", c=CH)
        mvb = cb16_t[:, _MV:_KVT].rearrange("p (c h) -> p c h", c=CH)
        kmat = consts.tile([P, CH, t_dim, H], BF16, tag="kmat")
        nc.vector.tensor_tensor(
            kmat,
            kT[:, :, :, None].to_broadcast((P, CH, t_dim, H)),
            m48b[:, :, None, :].to_broadcast((P, CH, t_dim, H)),
            op=mybir.AluOpType.mult,
        )
        vsel = consts.tile([P, CH, t_dim, H], BF16, tag="vsel")
        nc.vector.tensor_tensor(
            vsel,
            vT[:, :, :, None].to_broadcast((P, CH, t_dim, H)),
            mvb[:, :, None, :].to_broadcast((P, CH, t_dim, H)),
            op=mybir.AluOpType.mult,
        )

        # Wqk[d_in, t, h] = sum_dmid Wq[d_in, dmid] Kmat[dmid, t, h]
        wqk = consts.tile([P, CH, t_dim, H], BF16, tag="wqk")
        for ci in range(CH):
            pw = parp.tile([P, t_dim * H], F32, tag="par")
            for cm in range(CH):
                nc.tensor.matmul(
                    pw,
                    W(_WQT, cm, D)[:, ci * P : (ci + 1) * P],
                    kmat[:, cm, :, :],
                    start=(cm == 0), stop=(cm == CH - 1),
                )
            nc.vector.tensor_copy(wqk[:, ci], pw.rearrange("p (t h) -> p t h", h=H))

        # Wp_eff[(t,h), d_out] = sum_d Vsel[d, t, h] * Wp[d, d_out]
        pe_all = pop.tile([t_dim * H, D], F32, tag="po")
        for c in range(CH):
            nc.tensor.matmul(
                pe_all, vsel[:, c].rearrange("p t h -> p (t h)"), W(_WP, c, D),
                start=(c == 0), stop=(c == CH - 1),
            )
        wpe_stage = work.tile([t_dim * H, D], BF16, tag="wpestage")
        nc.scalar.copy(wpe_stage, pe_all)
        wpe_dram = dram.tile([t_dim * H, D], BF16)
        nc.sync.dma_start(wpe_dram, wpe_stage)
        wpe = consts.tile([H, t_dim, D], BF16, tag="wpe")
        nc.sync.dma_start(wpe, wpe_dram.rearrange("(t h) d -> h t d", h=H))
        # fold 1/|k_h| into the output projection
        wpe_s = consts.tile([H, t_dim, D], BF16, tag="wpes")
        nc.vector.tensor_tensor(
            wpe_s, wpe, rnkT[:, :, None].to_broadcast((H, t_dim, D)),
            op=mybir.AluOpType.mult,
        )

        # ---------- software-pipelined main loop ----------
        for g in range(PRE):
            emit_attn(g)
        for g in range(PRE - 1):
            emit_stageA(g)
        LAGB = 2
        for g in range(PRE, G):
            t, hf, gl = groups[g]
            if gl == 0 and (t, hf) not in qt_tiles:
                emit_qdma((t, hf))
            emit_qproj(g)
            emit_attn(g)
            emit_stageA(g - 1)
            if g - LAGB >= 0:
                emit_stageB(g - LAGB)
        emit_stageA(G - 1)
        for g in range(max(G - LAGB, 0), G):
            emit_stageB(g)

    nc.compile()
    return nc


_CACHE = {}


def _get_nc(t_dim=T, n_dim=N):
    key = (t_dim, n_dim, FP8_QPROJ, USE_DIVIDE)
    if key not in _CACHE:
        _CACHE[key] = build_nc(t_dim, n_dim)
    return _CACHE[key]


def _host_prep(query, kv, Wq, Wk, Wv, Wp, bp):
    bf = ml_dtypes.bfloat16
    f8 = ml_dtypes.float8_e4m3fn
    query = np.asarray(query, dtype=np.float32)
    kv = np.asarray(kv, dtype=np.float32)
    Wq = np.asarray(Wq, dtype=np.float32)
    Wk = np.asarray(Wk, dtype=np.float32)
    Wv = np.asarray(Wv, dtype=np.float32)
    Wp = np.asarray(Wp, dtype=np.float32)
    bp = np.asarray(bp, dtype=np.float32)

    b_dim, t_dim, n_dim, d = query.shape
    dh = d // H

    def img(mat, width):  # [D, width] -> [P, CH*width] chunk-major image
        return np.ascontiguousarray(
            mat.reshape(CH, P, width).transpose(1, 0, 2).reshape(P, CH * width)
        )

    perm = (np.arange(d) % H) * dh + np.arange(d) // H
    Wvp = Wv[:, perm]
    dd = np.arange(d)
    hh = np.arange(H)
    m48 = (dd[:, None] // dh == hh[None, :]).astype(np.float32)
    mv = (dd[:, None] % H == hh[None, :]).astype(np.float32)

    base = np.empty((P, _cb16_total(t_dim)), dtype=bf)
    base[:, _WQT:_WK] = img(np.ascontiguousarray(Wq.T), d).astype(bf)
    base[:, _WK:_WVP] = img(Wk, d).astype(bf)
    base[:, _WVP:_WP] = img(Wvp, d).astype(bf)
    base[:, _WP:_M48] = img(Wp, d).astype(bf)
    base[:, _M48:_MV] = img(m48, H).astype(bf)
    base[:, _MV:_KVT] = img(mv, H).astype(bf)

    m48pad = np.zeros((d, P), dtype=np.float32)
    m48pad[:, :H] = m48
    c8 = np.empty((P, CH * d + CH * P), dtype=f8)
    c8[:, : CH * d] = img(Wq, d).astype(f8)
    c8[:, CH * d :] = img(m48pad, P).astype(f8)

    bp_img = np.ascontiguousarray(bp.reshape(CH, P).T.astype(np.float32))

    in_maps = []
    for b in range(b_dim):
        cb16 = base.copy()
        cb16[:, _KVT:] = img(kv[b, :, 0, :].T, t_dim).astype(bf)
        qTb = np.ascontiguousarray(query[b].transpose(0, 2, 1))
        in_maps.append(
            {
                "qT": qTb.astype(bf),
                "qT8": qTb.astype(f8),
                "cb16": cb16,
                "c8": c8,
                "bp": bp_img,
            }
        )
    return in_maps, (b_dim, t_dim, n_dim, d)


def _gather(results, shape):
    b_dim, t_dim, n_dim, d = shape
    out = np.empty((b_dim, t_dim, n_dim, d), dtype=np.float32)
    for b in range(b_dim):
        out[b] = results[b]["outT"].astype(np.float32).transpose(0, 2, 1)
    return out


def kernel(query, kv, Wq, Wk, Wv, Wp, bp):
    from concourse.bass_utils import run_bass_kernel_spmd

    in_maps, shape = _host_prep(query, kv, Wq, Wk, Wv, Wp, bp)
    nc = _get_nc(shape[1], shape[2])
    res = run_bass_kernel_spmd(nc, in_maps, core_ids=list(range(len(in_maps))))
    return _gather(res.results, shape)


def _install_ntff_hook():
    """The agent image's antenv lacks axon_hooks; synthesize it so
    run_bass_kernel_spmd(trace=True) can capture NTFF profiles."""
    import types

    if "antenv.axon_hooks" in sys.modules:
        return
    sys.path.insert(0, "/root/.axon_site")
    from trn_agent_boot.trn_boot import _ntff_profile_via_ctypes

    hook = _ntff_profile_via_ctypes("/opt/axon/libaxon_pjrt.so")
    mod = types.ModuleType("antenv.axon_hooks")
    mod.get_axon_ntff_profile_hook = lambda: hook
    mod.set_axon_ntff_profile_hook = lambda h: None
    sys.modules["antenv.axon_hooks"] = mod


def kernel_traced(query, kv, Wq, Wk, Wv, Wp, bp):
    """Like kernel() but captures an NTFF profile; returns (out, results)."""
    from concourse.bass_utils import run_bass_kernel_spmd

    _install_ntff_hook()
    in_maps, shape = _host_prep(query, kv, Wq, Wk, Wv, Wp, bp)
    nc = _get_nc(shape[1], shape[2])
    res = run_bass_kernel_spmd(
        nc, in_maps, core_ids=list(range(len(in_maps))), trace=True
    )
    return _gather(res.results, shape), res

